# revision 16
# baseline (speedup 1.0000x reference)
"""Trainium2 Bass kernel for nn_HadamardTransform: out = value @ (weight + permutation).

Data-parallel over the 8192 token rows across 8 NeuronCores (1024 rows/core).
Everything runs in the transposed frame:  o[n, m] = sum_k (H+P)[k,n] vT[k,m]
with H symmetric Sylvester (scaled 1/64) and P a one-hot permutation, so
o = H vT + vT[src, :] where src[n] = argmax_k P[k, n].

Structured path (v3):
  H_4096 = H_8 (x) H_512  (Kronecker, i = i1*512 + i0).
  - PE: per 512-block i1, u_{i1} = (H_512/64) v_{i1}  (bf16 matmuls, fp32 PSUM,
    two 128-col groups share a PSUM bank pair -> one wide Act evacuation)
  - Act: PSUM -> SBUF bf16 evacuation
  - DVE: 3 radix-2 FWHT butterfly stages across the 8 blocks (bf16, all-SBUF)
  - Permutation term vT[src]: the row reorder is applied host-side as input
    prep (vP input); the add runs on device (DVE + GpSimd).  On-device
    indirect-DMA gather was measured 4.5x slower: all its traffic serializes
    through the single SWDGE queue (~22 GB/s).
  - All DRAM I/O uses HOST-PRE-TILED layouts ([partition, chunk, ...] with
    16KB contiguous runs per partition): descriptor-count-bound DMA measured
    ~76 GB/s with 1KB runs; 16KB runs are bandwidth-bound.
  - m processed in 4 chunks of 256 with all tile pools double-buffered, so
    chunks pipeline across PE/Act/DVE/DMA with no false serialization.
  - Outputs written bf16 on the Act HWDGE queue (parallel to SP loads);
    host casts back to fp32 and untiles.
bf16 is exact for H/64 and the butterflies; only value rounding contributes
error (~7e-3 relative vs the 1e-2 gate).
"""

import sys

sys.path.insert(0, "/opt/trn_rl_repo")

import numpy as np

import concourse.bacc as bacc
import concourse.bass as bass
import concourse.mybir as mybir
import concourse.tile as tile
from concourse.bass_utils import run_bass_kernel_spmd

ROWS = 8192
N = 4096
N_CORES = 8
MPC = ROWS // N_CORES  # 1024 token rows per core
KT = N // 128  # 32 k-tiles
NB = N // 128  # 32 n-blocks
MC = MPC // 512  # legacy (dense path m-chunks)

BF16 = mybir.dt.np(mybir.dt.bfloat16)

_cache = {}


# ---------------- structured (Hadamard) path ----------------

B = 512          # PE transform block size
KS = B // 128    # 4 k-subtiles per block
I1 = N // B      # 8 blocks -> 3 DVE butterfly stages
J2S = B // 128   # 4 output 128-row subblocks per block
MH = 256         # m chunk width
NH = MPC // MH   # 4 chunks
N_POOL_ADD = 4   # permutation-add blocks offloaded from DVE to GpSimd
UNROLL = 2       # reps emitted per For_i iteration (amortizes the barrier)


def _hadamard_pm1(n):
    idx = np.arange(n, dtype=np.int64)
    m = idx[:, None] & idx[None, :]
    pop = np.zeros_like(m)
    for _ in range(int(np.log2(n))):
        pop += m & 1
        m >>= 1
    return np.where(pop % 2 == 0, 1.0, -1.0).astype(np.float32)


def check_structure(weight, permutation):
    """weight must be the scaled Sylvester Hadamard, permutation one-hot."""
    H = _hadamard_pm1(N) / np.sqrt(np.float32(N))
    if not np.array_equal(weight, H):
        return None
    src = np.argmax(permutation, axis=0).astype(np.int32)
    ok = (
        permutation[src, np.arange(N)].min() == 1.0
        and permutation.sum() == N
        and np.abs(permutation).sum() == N
    )
    return src if ok else None


def build_hadamard(reps=1, hw_loop=False):
    nc = bacc.Bacc("TRN2", target_bir_lowering=False)
    # host-pre-tiled layouts: per (partition, chunk) runs are contiguous
    vT = nc.dram_tensor("vT", (128, NH, KT, MH), mybir.dt.bfloat16, kind="ExternalInput")
    vP = nc.dram_tensor("vP", (128, NH, NB, MH), mybir.dt.bfloat16, kind="ExternalInput")
    hb = nc.dram_tensor("hb", (B, B), mybir.dt.bfloat16, kind="ExternalInput")
    o = nc.dram_tensor("o", (128, NH, I1, J2S, MH), mybir.dt.bfloat16, kind="ExternalOutput")

    add, sub = mybir.AluOpType.add, mybir.AluOpType.subtract

    with tile.TileContext(nc) as tc:
        with (
            tc.tile_pool(name="hbp", bufs=1) as hb_pool,
            tc.tile_pool(name="vt", bufs=2) as vt_pool,
            tc.tile_pool(name="vp", bufs=2) as vp_pool,
            tc.tile_pool(name="ps", bufs=4, space="PSUM") as ps_pool,
            tc.tile_pool(name="u", bufs=2) as u_pool,
            tc.tile_pool(name="w", bufs=2) as w_pool,
            tc.tile_pool(name="t", bufs=2) as t_pool,
            tc.tile_pool(name="oo", bufs=2) as o_pool,
        ):
            # H_512/64 as lhsT panels: hbt[p, ks, j] = hb[ks*128+p, j]
            hbt = hb_pool.tile([128, KS, B], mybir.dt.bfloat16, tag="hbt")
            nc.sync.dma_start(
                out=hbt, in_=hb[:, :].rearrange("(ks p) j -> p ks j", p=128)
            )

            if hw_loop and reps > UNROLL:
                assert reps % UNROLL == 0
                loop_cm = tc.For_i(0, reps // UNROLL)
                loop_cm.__enter__()
                rep_range = range(UNROLL)
            else:
                loop_cm = None
                rep_range = range(reps)

            for rep in rep_range:
                for q in range(NH):
                    # 2MB input chunks, 16KB contiguous per partition
                    vts = vt_pool.tile([128, KT, MH], mybir.dt.bfloat16, tag="vts")
                    nc.sync.dma_start(out=vts, in_=vT[:, q, :, :])
                    vps = vp_pool.tile([128, NB, MH], mybir.dt.bfloat16, tag="vps")
                    nc.sync.dma_start(out=vps, in_=vP[:, q, :, :])

                    # PE: u_{i1}[j2s*128+p, m] = sum_ks (H/64)[ks-tile] v_{i1}
                    # two 128-col groups per PSUM tile -> one wide Act copy
                    us = []
                    for i1 in range(I1):
                        u = u_pool.tile([128, J2S, MH], mybir.dt.bfloat16, tag=f"u{i1}")
                        us.append(u)
                        for jp in range(J2S // 2):
                            ps = ps_pool.tile([128, 2 * MH], mybir.dt.float32, tag="ps")
                            for half in range(2):
                                j2s = 2 * jp + half
                                for ks in range(KS):
                                    nc.tensor.matmul(
                                        out=ps[:, half * MH : (half + 1) * MH],
                                        lhsT=hbt[:, ks, j2s * 128 : (j2s + 1) * 128],
                                        rhs=vts[:, i1 * KS + ks, :],
                                        start=(ks == 0),
                                        stop=(ks == KS - 1),
                                    )
                            nc.scalar.copy(
                                out=u[:, 2 * jp : 2 * jp + 2, :], in_=ps[:, :]
                            )

                    # DVE: 3 radix-2 FWHT stages across i1
                    ts = [
                        t_pool.tile([128, J2S, MH], mybir.dt.bfloat16, tag=f"t{i}", name=f"ts{i}")
                        for i in range(I1)
                    ]
                    for i in range(0, I1, 2):  # bit 0
                        nc.vector.tensor_tensor(out=ts[i], in0=us[i], in1=us[i + 1], op=add)
                        nc.vector.tensor_tensor(out=ts[i + 1], in0=us[i], in1=us[i + 1], op=sub)
                    ws = [
                        w_pool.tile([128, J2S, MH], mybir.dt.bfloat16, tag=f"w{i}", name=f"ws{i}")
                        for i in range(I1)
                    ]
                    for g in (0, 4):  # bit 1
                        for i in (g, g + 1):
                            nc.vector.tensor_tensor(out=ws[i], in0=ts[i], in1=ts[i + 2], op=add)
                            nc.vector.tensor_tensor(out=ws[i + 2], in0=ts[i], in1=ts[i + 2], op=sub)
                    # bit 2 -> single output tile [128, I1, J2S, MH]
                    oa = o_pool.tile([128, I1, J2S, MH], mybir.dt.bfloat16, tag="oa")
                    for i in range(4):
                        nc.vector.tensor_tensor(out=oa[:, i, :, :], in0=ws[i], in1=ws[i + 4], op=add)
                        nc.vector.tensor_tensor(out=oa[:, i + 4, :, :], in0=ws[i], in1=ws[i + 4], op=sub)

                    # permutation add (DVE for most blocks, GpSimd for a few)
                    for j1 in range(I1):
                        eng = nc.gpsimd if j1 >= I1 - N_POOL_ADD else nc.vector
                        eng.tensor_tensor(
                            out=oa[:, j1, :, :],
                            in0=oa[:, j1, :, :],
                            in1=vps[:, j1 * J2S : (j1 + 1) * J2S, :],
                            op=add,
                        )
                    # one output DMA per chunk on the Act HWDGE queue
                    nc.scalar.dma_start(out=o[:, q, :, :, :], in_=oa)

            if loop_cm is not None:
                loop_cm.__exit__(None, None, None)
    nc.compile()
    return nc


def make_in_maps_h(value, src):
    vTb = np.ascontiguousarray(value.T).astype(BF16)  # [N, ROWS]
    vPb = vTb[src]  # host-permuted rows: vP[n] = vT[src[n]]
    Hs = np.ascontiguousarray(_hadamard_pm1(B) / 64.0).astype(BF16)
    in_maps = []
    for c in range(N_CORES):
        sl = slice(c * MPC, (c + 1) * MPC)
        # [N, MPC] -> [128, NH, KT, MH]: row t*128+p, col q*MH+m -> [p, q, t, m]
        vt = np.ascontiguousarray(
            vTb[:, sl].reshape(KT, 128, NH, MH).transpose(1, 2, 0, 3)
        )
        vp = np.ascontiguousarray(
            vPb[:, sl].reshape(NB, 128, NH, MH).transpose(1, 2, 0, 3)
        )
        in_maps.append({"vT": vt, "vP": vp, "hb": Hs})
    return in_maps


def untile_out(o_tiled):
    """[128, NH, I1, J2S, MH] -> [N, MPC] (transposed frame)."""
    return np.ascontiguousarray(
        np.asarray(o_tiled).transpose(2, 3, 0, 1, 4).reshape(N, MPC)
    )


# ---------------- dense fallback (arbitrary weight/permutation) ----------------


def build_dense():
    nc = bacc.Bacc("TRN2", target_bir_lowering=False)
    vT = nc.dram_tensor("vT", (N, MPC), mybir.dt.float32r, kind="ExternalInput")
    wgt = nc.dram_tensor("wgt", (N, N), mybir.dt.float32, kind="ExternalInput")
    prm = nc.dram_tensor("prm", (N, N), mybir.dt.float32, kind="ExternalInput")
    o = nc.dram_tensor("o", (N, MPC), mybir.dt.float32, kind="ExternalOutput")

    with tile.TileContext(nc) as tc:
        with (
            tc.tile_pool(name="vt", bufs=1) as vt_pool,
            tc.tile_pool(name="wp", bufs=2) as wp_pool,
            tc.tile_pool(name="pp", bufs=2) as pp_pool,
            tc.tile_pool(name="ps", bufs=4, space="PSUM") as ps_pool,
            tc.tile_pool(name="os", bufs=4) as os_pool,
        ):
            vts = []
            for t in range(KT):
                vt_t = vt_pool.tile([128, MPC], mybir.dt.float32r, tag=f"vt{t}")
                nc.sync.dma_start(out=vt_t, in_=vT[t * 128 : (t + 1) * 128, :])
                vts.append(vt_t)

            for nb in range(NB):
                n0 = nb * 128
                wp = wp_pool.tile([128, KT, 128], mybir.dt.float32r, tag="wp")
                pp = pp_pool.tile([128, KT, 128], mybir.dt.float32, tag="pp")
                wsrc = wgt[:, n0 : n0 + 128].rearrange("(kt p) j -> p kt j", p=128)
                psrc = prm[:, n0 : n0 + 128].rearrange("(kt p) j -> p kt j", p=128)
                nc.sync.dma_start(out=wp[:, :, :].bitcast(mybir.dt.float32), in_=wsrc)
                nc.sync.dma_start(out=pp, in_=psrc)
                nc.vector.tensor_tensor(
                    out=wp[:, :, :],
                    in0=wp[:, :, :].bitcast(mybir.dt.float32),
                    in1=pp[:, :, :],
                    op=mybir.AluOpType.add,
                )
                for mc in range(MC):
                    ps = ps_pool.tile([128, 512], mybir.dt.float32, tag="ps")
                    for kt in range(KT):
                        nc.tensor.matmul(
                            out=ps[:, :],
                            lhsT=wp[:, kt, :],
                            rhs=vts[kt][:, mc * 512 : (mc + 1) * 512],
                            start=(kt == 0),
                            stop=(kt == KT - 1),
                        )
                    ot = os_pool.tile([128, 512], mybir.dt.float32, tag="os")
                    nc.scalar.copy(out=ot[:, :], in_=ps[:, :])
                    nc.sync.dma_start(
                        out=o[n0 : n0 + 128, mc * 512 : (mc + 1) * 512], in_=ot
                    )
    nc.compile()
    return nc


def make_in_maps(value, weight, permutation):
    vT = np.ascontiguousarray(value.T)  # [N, ROWS]
    w = np.ascontiguousarray(weight, dtype=np.float32)
    p = np.ascontiguousarray(permutation, dtype=np.float32)
    in_maps = []
    for c in range(N_CORES):
        in_maps.append(
            {
                "vT": np.ascontiguousarray(vT[:, c * MPC : (c + 1) * MPC]),
                "wgt": w,
                "prm": p,
            }
        )
    return in_maps


def kernel(value, weight, permutation):
    value = np.asarray(value, dtype=np.float32)
    weight = np.asarray(weight, dtype=np.float32)
    permutation = np.asarray(permutation, dtype=np.float32)
    src = check_structure(weight, permutation)
    if src is not None:
        if "had" not in _cache:
            _cache["had"] = build_hadamard()
        nc = _cache["had"]
        in_maps = make_in_maps_h(value, src)
        res = run_bass_kernel_spmd(nc, in_maps, core_ids=list(range(N_CORES)))
        out = np.concatenate(
            [
                untile_out(res.results[c]["o"]).T.astype(np.float32)
                for c in range(N_CORES)
            ],
            axis=0,
        )
        return out
    if "dense" not in _cache:
        _cache["dense"] = build_dense()
    nc = _cache["dense"]
    in_maps = make_in_maps(value, weight, permutation)
    res = run_bass_kernel_spmd(nc, in_maps, core_ids=list(range(N_CORES)))
    out = np.concatenate(
        [np.ascontiguousarray(res.results[c]["o"].T) for c in range(N_CORES)], axis=0
    )
    return out
